# revision 6
# baseline (speedup 1.0000x reference)
"""Trainium2 Bass kernel for nn_CachedMLP (2-expert cached MoE MLP).

Math (per reference.py): for each expert e in {0,1}
    u_e = (h @ w3_e.T)[:, idx]  ==  h @ (w3_e[idx, :]).T      (column gather == row gather on w3)
    g_e = silu(h @ w1_e.T)
    out = sum_e ew_e * ((g_e * u_e) @ w2_e)

Strategy (memory-bound problem, ~1.2 GB fp32 of weights, 32 tokens):
  * Host: apply the index gather to w3 rows, fold the routing scalars ew_e
    into w2, pad ACTIVE 11468 -> 11472, shard all three weight matrices
    along the ACTIVE axis across 8 NeuronCores, cast to fp16 (halves HBM
    traffic; fp16 keeps ~5e-4 matmul rounding vs bf16's ~4e-3).
  * Device (per core), per (expert, 128-row ACTIVE chunk):
      - DMA one contraction slab (w3gT and w1T k-chunks) and the w2 row
        strip; 32+32 accumulating matmuls -> uT/gT [mw, 32] in PSUM
        (one accumulation group per bank at a time — HW `start` clears
        has_written for the whole bank);
      - silu (ACT) * mul (DVE) -> pT [mw, 32] fp16;
      - 32 single-shot matmuls w2-chunk.T @ pT -> outT n-chunks [128, 32],
        16 per scratch bank (sequential groups; data persists after stop);
      - DVE-accumulate the two scratch banks into an SBUF outT accumulator.
  * Host: un-transpose and sum the 8 per-core partials (no device
    collectives needed).

kernel(**inputs) takes the full unsharded inputs and returns the full
[32, 4096] fp32 output.
"""

import numpy as np

import concourse.bass as bass
import concourse.mybir as mybir
import concourse.tile as tile
from concourse import bacc
from concourse.bass_utils import run_bass_kernel_spmd

NCORES = 8
T = 32              # tokens
D = 4096            # d_model
HIDDEN = 14336
ACTIVE = 11468
A_PAD = 11472       # ACTIVE padded to a multiple of NCORES
AC = A_PAD // NCORES          # 1434 ACTIVE-rows per core
MCH = (AC + 127) // 128       # 12 chunks of <=128 rows (last chunk = 26)
KCH = D // 128                # 32 contraction chunks over d_model
FD = mybir.dt.float16
F32 = mybir.dt.float32

# column widths/offsets of the per-(e,m) slabs inside the packed wug tensor
_SLAB_W = [2 * KCH * min(128, AC - m * 128) for m in range(MCH)]
_SLAB_OFF = {}
_off = 0
for _e in range(2):
    for _m in range(MCH):
        _SLAB_OFF[(_e, _m)] = _off
        _off += _SLAB_W[_m]
WUG_COLS = _off  # 2 * 2*KCH*AC = 183552

_CACHE: dict = {}


def build_program() -> bass.Bass:
    nc = bacc.Bacc("TRN2", target_bir_lowering=False, debug=False, num_devices=NCORES)

    h_in = nc.dram_tensor("h", [128, KCH * T], FD, kind="ExternalInput")
    # wug[p, SLAB_OFF(e,m) + which*KCH*mw + k*mw + j] = W.T[k*128 + p, m*128 + j]
    #   W = w3_gathered_e (which=0) or w1_e (which=1), rows local to this shard
    wug = nc.dram_tensor("wug", [128, WUG_COLS], FD, kind="ExternalInput")
    w2 = nc.dram_tensor("w2", [2, AC, D], FD, kind="ExternalInput")
    # out[p, b*512 + nl*32 + t] = outT[(b*16+nl)*128 + p, t]  (partial over shard)
    out = nc.dram_tensor("out", [128, 1024], F32, kind="ExternalOutput")

    AF = mybir.ActivationFunctionType

    with tile.TileContext(nc) as tc:
        with (
            tc.tile_pool(name="hp", bufs=1) as hp,
            tc.tile_pool(name="slabs", bufs=3) as slabs,
            tc.tile_pool(name="w2pool", bufs=3) as w2pool,
            tc.tile_pool(name="ptp", bufs=3) as ptp,
            tc.tile_pool(name="silp", bufs=3) as silp,
            tc.tile_pool(name="obp", bufs=1) as obp,
            tc.tile_pool(name="pug", bufs=2, space="PSUM") as pug,
            tc.tile_pool(name="pos", bufs=2, space="PSUM") as pos,
        ):
            ht = hp.tile([128, KCH * T], FD, name="ht")
            nc.sync.dma_start(ht[:], h_in[:])

            osb = obp.tile([128, 1024], F32, name="osb")
            nc.gpsimd.memset(osb[:], 0.0)

            for e in range(2):
                for m in range(MCH):
                    mw = min(128, AC - m * 128)
                    w = _SLAB_W[m]
                    off = _SLAB_OFF[(e, m)]

                    sl = slabs.tile([128, 2 * KCH * 128], FD,
                                    name=f"sl{e}_{m}", tag="slab")
                    nc.sync.dma_start(sl[:, :w], wug[:, off:off + w])

                    w2t = w2pool.tile([128, D], FD, name=f"w2_{e}_{m}", tag="w2t")
                    nc.sync.dma_start(w2t[:mw], w2[e, m * 128: m * 128 + mw, :])

                    accu = pug.tile([128, T], F32, name=f"au{e}_{m}", tag="accu")
                    accg = pug.tile([128, T], F32, name=f"ag{e}_{m}", tag="accg")
                    for k in range(KCH):
                        nc.tensor.matmul(
                            accu[:mw],
                            lhsT=sl[:, k * mw:(k + 1) * mw],
                            rhs=ht[:, k * T:(k + 1) * T],
                            start=(k == 0), stop=(k == KCH - 1),
                        )
                    for k in range(KCH):
                        nc.tensor.matmul(
                            accg[:mw],
                            lhsT=sl[:, KCH * mw + k * mw: KCH * mw + (k + 1) * mw],
                            rhs=ht[:, k * T:(k + 1) * T],
                            start=(k == 0), stop=(k == KCH - 1),
                        )

                    # silu(g) = g * sigmoid(g); Sigmoid is HW-LUT'd and CoreSim-implemented
                    sig = silp.tile([128, T], F32, name=f"sig{e}_{m}", tag="sig")
                    nc.scalar.activation(sig[:mw], accg[:mw], AF.Sigmoid)
                    sil = silp.tile([128, T], F32, name=f"sil{e}_{m}", tag="sil")
                    nc.vector.tensor_mul(sil[:mw], sig[:mw], accg[:mw])
                    pt = ptp.tile([128, T], FD, name=f"pt{e}_{m}", tag="pt")
                    nc.vector.tensor_mul(pt[:mw], sil[:mw], accu[:mw])

                    # outT chunks: 16 sequential single-shot groups per bank
                    for b in range(2):
                        osc = pos.tile([128, 512], F32, name=f"os{e}_{m}_{b}",
                                       tag=f"osc{b}")
                        for nl in range(16):
                            n = b * 16 + nl
                            nc.tensor.matmul(
                                osc[:, nl * T:(nl + 1) * T],
                                lhsT=w2t[:mw, n * 128:(n + 1) * 128],
                                rhs=pt[:mw],
                                start=True, stop=True,
                            )
                        nc.vector.tensor_add(
                            osb[:, b * 512:(b + 1) * 512],
                            osb[:, b * 512:(b + 1) * 512],
                            osc[:],
                        )

            nc.sync.dma_start(out[:], osb[:])

    nc.compile()
    return nc


def get_program() -> bass.Bass:
    if "nc" not in _CACHE:
        _CACHE["nc"] = build_program()
    return _CACHE["nc"]


def prepare_in_maps(
    hidden_states, w3_0, w3_1, w1_0, w2_0, w1_1, w2_1,
    expert_weights, indices0, expert_ids,
) -> list[dict]:
    h = np.asarray(hidden_states, dtype=np.float32)
    ew = np.asarray(expert_weights, dtype=np.float32)
    eid = np.asarray(expert_ids)
    swap = bool(eid[0] != 0)
    ew0 = float(ew[1] if swap else ew[0])
    ew1 = float(ew[0] if swap else ew[1])

    idx = np.asarray(indices0).astype(np.int64)
    idxp = np.concatenate([idx, np.zeros(A_PAD - idx.shape[0], np.int64)])

    def prep_expert(w3, w1, w2, scale):
        w3g = np.asarray(w3, np.float32)[idxp].astype(np.float16)   # [A_PAD, D]
        w1p = np.zeros((A_PAD, D), np.float16)
        w1p[:ACTIVE] = np.asarray(w1, np.float32).astype(np.float16)
        w2p = np.zeros((A_PAD, D), np.float16)
        w2p[:ACTIVE] = (np.asarray(w2, np.float32) * scale).astype(np.float16)
        return w3g, w1p, w2p

    w3g0, w1p0, w2p0 = prep_expert(w3_0, w1_0, w2_0, ew0)
    w3g1, w1p1, w2p1 = prep_expert(w3_1, w1_1, w2_1, ew1)

    hT = np.ascontiguousarray(
        h.T.astype(np.float16).reshape(KCH, 128, T).transpose(1, 0, 2).reshape(128, KCH * T)
    )

    def slab(Wrows):  # [mw, D] -> [128, KCH*mw] with cols k*mw+j
        mw = Wrows.shape[0]
        return Wrows.T.reshape(KCH, 128, mw).transpose(1, 0, 2).reshape(128, KCH * mw)

    in_maps = []
    for c in range(NCORES):
        wug_c = np.empty((128, WUG_COLS), np.float16)
        for e, (w3g, w1p) in enumerate(((w3g0, w1p0), (w3g1, w1p1))):
            for m in range(MCH):
                mw = min(128, AC - m * 128)
                off = _SLAB_OFF[(e, m)]
                r = slice(c * AC + m * 128, c * AC + m * 128 + mw)
                wug_c[:, off: off + KCH * mw] = slab(w3g[r])
                wug_c[:, off + KCH * mw: off + 2 * KCH * mw] = slab(w1p[r])
        r = slice(c * AC, (c + 1) * AC)
        w2_c = np.ascontiguousarray(np.stack([w2p0[r], w2p1[r]]))  # [2, AC, D]
        in_maps.append({"h": hT, "wug": wug_c, "w2": w2_c})
    return in_maps


def reduce_outputs(results: list[dict]) -> np.ndarray:
    total = np.zeros((T, D), np.float64)
    for res in results:
        x = np.asarray(res["out"])                    # [128, 1024] f32
        total += x.reshape(128, 2, 16, T).transpose(3, 1, 2, 0).reshape(T, D)
    return total.astype(np.float32)


def run_spmd(in_maps, **kwargs):
    nc = get_program()
    return run_bass_kernel_spmd(nc, in_maps, core_ids=list(range(NCORES)), **kwargs)


def kernel(**inputs) -> np.ndarray:
    in_maps = prepare_in_maps(**inputs)
    res = run_spmd(in_maps)
    return reduce_outputs(res.results)


# revision 7
# speedup vs baseline: 2.4080x; 2.4080x over previous
"""Trainium2 Bass kernel for nn_CachedMLP (2-expert cached MoE MLP).

Math (per reference.py): for each expert e in {0,1}
    u_e = (h @ w3_e.T)[:, idx]  ==  h @ (w3_e[idx, :]).T      (column gather == row gather on w3)
    g_e = silu(h @ w1_e.T)
    out = sum_e ew_e * ((g_e * u_e) @ w2_e)

Strategy (memory-bound problem, ~1.2 GB fp32 of weights, 32 tokens):
  * Host: apply the index gather to w3 rows, fold the routing scalars ew_e
    into w2, pad ACTIVE 11468 -> 11472, shard all three weight matrices
    along the ACTIVE axis across 8 NeuronCores, cast to fp16 (halves HBM
    traffic; fp16 keeps ~5e-4 matmul rounding vs bf16's ~4e-3).
  * Device (per core), per (expert, 128-row ACTIVE chunk):
      - DMA one contraction slab (w3gT and w1T k-chunks) and the w2 row
        strip; 32+32 accumulating matmuls -> uT/gT [mw, 32] in PSUM
        (one accumulation group per bank at a time — HW `start` clears
        has_written for the whole bank);
      - silu (ACT) * mul (DVE) -> pT [mw, 32] fp16;
      - 32 single-shot matmuls w2-chunk.T @ pT -> outT n-chunks [128, 32],
        16 per scratch bank (sequential groups; data persists after stop);
      - DVE-accumulate the two scratch banks into an SBUF outT accumulator.
  * Host: un-transpose and sum the 8 per-core partials (no device
    collectives needed).

kernel(**inputs) takes the full unsharded inputs and returns the full
[32, 4096] fp32 output.
"""

import numpy as np

import concourse.bass as bass
import concourse.mybir as mybir
import concourse.tile as tile
from concourse import bacc
from concourse.bass_utils import run_bass_kernel_spmd

NCORES = 8
T = 32              # tokens
D = 4096            # d_model
HIDDEN = 14336
ACTIVE = 11468
A_PAD = 11472       # ACTIVE padded to a multiple of NCORES
AC = A_PAD // NCORES          # 1434 ACTIVE-rows per core
MCH = (AC + 127) // 128       # 12 chunks of <=128 rows (last chunk = 26)
KCH = D // 128                # 32 contraction chunks over d_model
FD = mybir.dt.float16
F32 = mybir.dt.float32

# column widths/offsets of the per-(e,m) slabs inside the packed wug tensor
_SLAB_W = [2 * KCH * min(128, AC - m * 128) for m in range(MCH)]
_SLAB_OFF = {}
_off = 0
for _e in range(2):
    for _m in range(MCH):
        _SLAB_OFF[(_e, _m)] = _off
        _off += _SLAB_W[_m]
WUG_COLS = _off  # 2 * 2*KCH*AC = 183552

_CACHE: dict = {}


def build_program(reps: int = 1) -> bass.Bass:
    nc = bacc.Bacc("TRN2", target_bir_lowering=False, debug=False, num_devices=NCORES)

    h_in = nc.dram_tensor("h", [128, KCH * T], FD, kind="ExternalInput")
    # wug[p, SLAB_OFF(e,m) + which*KCH*mw + k*mw + j] = W.T[k*128 + p, m*128 + j]
    #   W = w3_gathered_e (which=0) or w1_e (which=1), rows local to this shard
    wug = nc.dram_tensor("wug", [128, WUG_COLS], FD, kind="ExternalInput")
    w2 = nc.dram_tensor("w2", [2, AC, D], FD, kind="ExternalInput")
    # out[p, b*512 + nl*32 + t] = outT[(b*16+nl)*128 + p, t]  (partial over shard)
    out = nc.dram_tensor("out", [128, 1024], F32, kind="ExternalOutput")

    AF = mybir.ActivationFunctionType

    with tile.TileContext(nc) as tc:
        with (
            tc.tile_pool(name="hp", bufs=1) as hp,
            tc.tile_pool(name="slabs", bufs=3) as slabs,
            tc.tile_pool(name="w2pool", bufs=3) as w2pool,
            tc.tile_pool(name="ptp", bufs=3) as ptp,
            tc.tile_pool(name="silp", bufs=3) as silp,
            tc.tile_pool(name="obp", bufs=2) as obp,
            tc.tile_pool(name="pug", bufs=2, space="PSUM") as pug,
            tc.tile_pool(name="pos", bufs=2, space="PSUM") as pos,
        ):
            ht = hp.tile([128, KCH * T], FD, name="ht")
            nc.sync.dma_start(ht[:], h_in[:])

            for rep in range(reps):
                osb = obp.tile([128, 1024], F32, name=f"osb{rep}", tag="osb")
                nc.gpsimd.memset(osb[:], 0.0)

                for e in range(2):
                    for m in range(MCH):
                        mw = min(128, AC - m * 128)
                        w = _SLAB_W[m]
                        off = _SLAB_OFF[(e, m)]

                        sl = slabs.tile([128, 2 * KCH * 128], FD,
                                        name=f"sl{rep}_{e}_{m}", tag="slab")
                        nc.sync.dma_start(sl[:, :w], wug[:, off:off + w])

                        w2t = w2pool.tile([128, D], FD,
                                          name=f"w2_{rep}_{e}_{m}", tag="w2t")
                        nc.sync.dma_start(w2t[:mw], w2[e, m * 128: m * 128 + mw, :])

                        accu = pug.tile([128, T], F32, name=f"au{rep}_{e}_{m}",
                                        tag="accu")
                        accg = pug.tile([128, T], F32, name=f"ag{rep}_{e}_{m}",
                                        tag="accg")
                        for k in range(KCH):
                            nc.tensor.matmul(
                                accu[:mw],
                                lhsT=sl[:, k * mw:(k + 1) * mw],
                                rhs=ht[:, k * T:(k + 1) * T],
                                start=(k == 0), stop=(k == KCH - 1),
                            )
                        for k in range(KCH):
                            nc.tensor.matmul(
                                accg[:mw],
                                lhsT=sl[:, KCH * mw + k * mw: KCH * mw + (k + 1) * mw],
                                rhs=ht[:, k * T:(k + 1) * T],
                                start=(k == 0), stop=(k == KCH - 1),
                            )

                        # silu(g) = g * sigmoid(g); Sigmoid is HW-LUT'd and
                        # implemented in CoreSim (Silu is not)
                        sig = silp.tile([128, T], F32, name=f"sig{rep}_{e}_{m}",
                                        tag="sig")
                        nc.scalar.activation(sig[:mw], accg[:mw], AF.Sigmoid)
                        sil = silp.tile([128, T], F32, name=f"sil{rep}_{e}_{m}",
                                        tag="sil")
                        nc.vector.tensor_mul(sil[:mw], sig[:mw], accg[:mw])
                        pt = ptp.tile([128, T], FD, name=f"pt{rep}_{e}_{m}", tag="pt")
                        nc.vector.tensor_mul(pt[:mw], sil[:mw], accu[:mw])

                        # outT chunks: 16 sequential single-shot groups per bank
                        for b in range(2):
                            osc = pos.tile([128, 512], F32,
                                           name=f"os{rep}_{e}_{m}_{b}", tag=f"osc{b}")
                            for nl in range(16):
                                n = b * 16 + nl
                                nc.tensor.matmul(
                                    osc[:, nl * T:(nl + 1) * T],
                                    lhsT=w2t[:mw, n * 128:(n + 1) * 128],
                                    rhs=pt[:mw],
                                    start=True, stop=True,
                                )
                            nc.vector.tensor_add(
                                osb[:, b * 512:(b + 1) * 512],
                                osb[:, b * 512:(b + 1) * 512],
                                osc[:],
                            )

                nc.sync.dma_start(out[:], osb[:])

    nc.compile()
    return nc


def get_program(reps: int = 1) -> bass.Bass:
    key = ("nc", reps)
    if key not in _CACHE:
        _CACHE[key] = build_program(reps)
    return _CACHE[key]


def prepare_in_maps(
    hidden_states, w3_0, w3_1, w1_0, w2_0, w1_1, w2_1,
    expert_weights, indices0, expert_ids,
) -> list[dict]:
    h = np.asarray(hidden_states, dtype=np.float32)
    ew = np.asarray(expert_weights, dtype=np.float32)
    eid = np.asarray(expert_ids)
    swap = bool(eid[0] != 0)
    ew0 = float(ew[1] if swap else ew[0])
    ew1 = float(ew[0] if swap else ew[1])

    idx = np.asarray(indices0).astype(np.int64)
    idxp = np.concatenate([idx, np.zeros(A_PAD - idx.shape[0], np.int64)])

    def prep_expert(w3, w1, w2, scale):
        w3g = np.asarray(w3, np.float32)[idxp].astype(np.float16)   # [A_PAD, D]
        w1p = np.zeros((A_PAD, D), np.float16)
        w1p[:ACTIVE] = np.asarray(w1, np.float32).astype(np.float16)
        w2p = np.zeros((A_PAD, D), np.float16)
        w2p[:ACTIVE] = (np.asarray(w2, np.float32) * scale).astype(np.float16)
        return w3g, w1p, w2p

    w3g0, w1p0, w2p0 = prep_expert(w3_0, w1_0, w2_0, ew0)
    w3g1, w1p1, w2p1 = prep_expert(w3_1, w1_1, w2_1, ew1)

    hT = np.ascontiguousarray(
        h.T.astype(np.float16).reshape(KCH, 128, T).transpose(1, 0, 2).reshape(128, KCH * T)
    )

    def slab(Wrows):  # [mw, D] -> [128, KCH*mw] with cols k*mw+j
        mw = Wrows.shape[0]
        return Wrows.T.reshape(KCH, 128, mw).transpose(1, 0, 2).reshape(128, KCH * mw)

    in_maps = []
    for c in range(NCORES):
        wug_c = np.empty((128, WUG_COLS), np.float16)
        for e, (w3g, w1p) in enumerate(((w3g0, w1p0), (w3g1, w1p1))):
            for m in range(MCH):
                mw = min(128, AC - m * 128)
                off = _SLAB_OFF[(e, m)]
                r = slice(c * AC + m * 128, c * AC + m * 128 + mw)
                wug_c[:, off: off + KCH * mw] = slab(w3g[r])
                wug_c[:, off + KCH * mw: off + 2 * KCH * mw] = slab(w1p[r])
        r = slice(c * AC, (c + 1) * AC)
        w2_c = np.ascontiguousarray(np.stack([w2p0[r], w2p1[r]]))  # [2, AC, D]
        in_maps.append({"h": hT, "wug": wug_c, "w2": w2_c})
    return in_maps


def reduce_outputs(results: list[dict]) -> np.ndarray:
    total = np.zeros((T, D), np.float64)
    for res in results:
        x = np.asarray(res["out"])                    # [128, 1024] f32
        total += x.reshape(128, 2, 16, T).transpose(3, 1, 2, 0).reshape(T, D)
    return total.astype(np.float32)


def run_spmd(in_maps, **kwargs):
    nc = get_program()
    return run_bass_kernel_spmd(nc, in_maps, core_ids=list(range(NCORES)), **kwargs)


def kernel(**inputs) -> np.ndarray:
    in_maps = prepare_in_maps(**inputs)
    res = run_spmd(in_maps)
    return reduce_outputs(res.results)


# revision 9
# speedup vs baseline: 2.7135x; 1.1268x over previous
"""Trainium2 Bass kernel for nn_CachedMLP (2-expert cached MoE MLP).

Math (per reference.py): for each expert e in {0,1}
    u_e = (h @ w3_e.T)[:, idx]  ==  h @ (w3_e[idx, :]).T      (column gather == row gather on w3)
    g_e = silu(h @ w1_e.T)
    out = sum_e ew_e * ((g_e * u_e) @ w2_e)

Strategy (memory-bound problem, ~1.2 GB fp32 of weights, 32 tokens):
  * Host: apply the index gather to w3 rows, fold the routing scalars ew_e
    into w2, pad ACTIVE 11468 -> 11472, shard all three weight matrices
    along the ACTIVE axis across 8 NeuronCores, cast to fp16 (halves HBM
    traffic; fp16 keeps ~5e-4 matmul rounding vs bf16's ~4e-3).
  * Device (per core), per (expert, 128-row ACTIVE chunk):
      - DMA one contraction slab (w3gT and w1T k-chunks) and the w2 row
        strip; 32+32 accumulating matmuls -> uT/gT [mw, 32] in PSUM
        (one accumulation group per bank at a time — HW `start` clears
        has_written for the whole bank);
      - silu (ACT) * mul (DVE) -> pT [mw, 32] fp16;
      - 32 single-shot matmuls w2-chunk.T @ pT -> outT n-chunks [128, 32],
        16 per scratch bank (sequential groups; data persists after stop);
      - DVE-accumulate the two scratch banks into an SBUF outT accumulator.
  * Host: un-transpose and sum the 8 per-core partials (no device
    collectives needed).

kernel(**inputs) takes the full unsharded inputs and returns the full
[32, 4096] fp32 output.
"""

import numpy as np

import concourse.bass as bass
import concourse.mybir as mybir
import concourse.tile as tile
from concourse import bacc
from concourse.bass_utils import run_bass_kernel_spmd

NCORES = 8
T = 32              # tokens
D = 4096            # d_model
HIDDEN = 14336
ACTIVE = 11468
A_PAD = 11472       # ACTIVE padded to a multiple of NCORES
AC = A_PAD // NCORES          # 1434 ACTIVE-rows per core
MCH = (AC + 127) // 128       # 12 chunks of <=128 rows (last chunk = 26)
KCH = D // 128                # 32 contraction chunks over d_model
FD = mybir.dt.float16
F32 = mybir.dt.float32

# column widths/offsets of the per-(e,m) slabs inside the packed wug tensor
_SLAB_W = [2 * KCH * min(128, AC - m * 128) for m in range(MCH)]
_SLAB_OFF = {}
_off = 0
for _e in range(2):
    for _m in range(MCH):
        _SLAB_OFF[(_e, _m)] = _off
        _off += _SLAB_W[_m]
WUG_COLS = _off  # 2 * 2*KCH*AC = 183552

_CACHE: dict = {}


def build_program(reps: int = 1) -> bass.Bass:
    nc = bacc.Bacc("TRN2", target_bir_lowering=False, debug=False, num_devices=NCORES)

    h_in = nc.dram_tensor("h", [128, KCH * T], FD, kind="ExternalInput")
    # wug[p, SLAB_OFF(e,m) + which*KCH*mw + k*mw + j] = W.T[k*128 + p, m*128 + j]
    #   W = w3_gathered_e (which=0) or w1_e (which=1), rows local to this shard
    wug = nc.dram_tensor("wug", [128, WUG_COLS], FD, kind="ExternalInput")
    w2 = nc.dram_tensor("w2", [2, AC, D], FD, kind="ExternalInput")
    # out[p, b*512 + nl*32 + t] = outT[(b*16+nl)*128 + p, t]  (partial over shard)
    out = nc.dram_tensor("out", [128, 1024], F32, kind="ExternalOutput")

    AF = mybir.ActivationFunctionType

    KH = KCH // 2  # k-chunks per slab half

    with tile.TileContext(nc) as tc:
        with (
            tc.tile_pool(name="hp", bufs=1) as hp,
            tc.tile_pool(name="slabs", bufs=6) as slabs,
            tc.tile_pool(name="w2pool", bufs=6) as w2pool,
            tc.tile_pool(name="ptp", bufs=3) as ptp,
            tc.tile_pool(name="silp", bufs=3) as silp,
            tc.tile_pool(name="obp", bufs=2) as obp,
            tc.tile_pool(name="pug", bufs=2, space="PSUM") as pug,
            tc.tile_pool(name="pos", bufs=2, space="PSUM") as pos,
        ):
            ht = hp.tile([128, KCH * T], FD, name="ht")
            nc.sync.dma_start(ht[:], h_in[:])

            for rep in range(reps):
                osb = obp.tile([128, 1024], F32, name=f"osb{rep}", tag="osb")
                nc.gpsimd.memset(osb[:], 0.0)

                for e in range(2):
                    for m in range(MCH):
                        mw = min(128, AC - m * 128)
                        off = _SLAB_OFF[(e, m)]

                        # two slab halves: [u k-half | g k-half], 1 DMA each
                        sls = []
                        for hh in range(2):
                            slh = slabs.tile([128, KCH * 128], FD,
                                             name=f"sl{rep}_{e}_{m}_{hh}", tag="slab")
                            nc.sync.dma_start(
                                slh[:, : KCH * mw],
                                wug[:, off + hh * KCH * mw: off + (hh + 1) * KCH * mw],
                            )
                            sls.append(slh)

                        # w2 column halves, one per outT scratch bank, on the
                        # second HWDGE ring (scalar) for queue parallelism
                        w2h = []
                        for b in range(2):
                            w2t = w2pool.tile([128, D // 2], FD,
                                              name=f"w2_{rep}_{e}_{m}_{b}",
                                              tag=f"w2t{b}")
                            nc.scalar.dma_start(
                                w2t[:mw],
                                w2[e, m * 128: m * 128 + mw,
                                   b * (D // 2): (b + 1) * (D // 2)],
                            )
                            w2h.append(w2t)

                        accu = pug.tile([128, T], F32, name=f"au{rep}_{e}_{m}",
                                        tag="accu")
                        accg = pug.tile([128, T], F32, name=f"ag{rep}_{e}_{m}",
                                        tag="accg")
                        for hh in range(2):
                            for which, acc in ((0, accu), (1, accg)):
                                for kl in range(KH):
                                    k = hh * KH + kl
                                    nc.tensor.matmul(
                                        acc[:mw],
                                        lhsT=sls[hh][:, (which * KH + kl) * mw:
                                                     (which * KH + kl + 1) * mw],
                                        rhs=ht[:, k * T:(k + 1) * T],
                                        start=(k == 0), stop=(k == KCH - 1),
                                    )

                        # silu(g) = g * sigmoid(g); Sigmoid is HW-LUT'd and
                        # implemented in CoreSim (Silu is not)
                        sig = silp.tile([128, T], F32, name=f"sig{rep}_{e}_{m}",
                                        tag="sig")
                        nc.scalar.activation(sig[:mw], accg[:mw], AF.Sigmoid)
                        sil = silp.tile([128, T], F32, name=f"sil{rep}_{e}_{m}",
                                        tag="sil")
                        nc.vector.tensor_mul(sil[:mw], sig[:mw], accg[:mw])
                        pt = ptp.tile([128, T], FD, name=f"pt{rep}_{e}_{m}", tag="pt")
                        nc.vector.tensor_mul(pt[:mw], sil[:mw], accu[:mw])

                        # outT chunks: 16 sequential single-shot groups per bank
                        for b in range(2):
                            osc = pos.tile([128, 512], F32,
                                           name=f"os{rep}_{e}_{m}_{b}", tag=f"osc{b}")
                            for nl in range(16):
                                nc.tensor.matmul(
                                    osc[:, nl * T:(nl + 1) * T],
                                    lhsT=w2h[b][:mw, nl * 128:(nl + 1) * 128],
                                    rhs=pt[:mw],
                                    start=True, stop=True,
                                )
                            nc.vector.tensor_add(
                                osb[:, b * 512:(b + 1) * 512],
                                osb[:, b * 512:(b + 1) * 512],
                                osc[:],
                            )

                nc.sync.dma_start(out[:], osb[:])

    nc.compile()
    return nc


def get_program(reps: int = 1) -> bass.Bass:
    key = ("nc", reps)
    if key not in _CACHE:
        _CACHE[key] = build_program(reps)
    return _CACHE[key]


def prepare_in_maps(
    hidden_states, w3_0, w3_1, w1_0, w2_0, w1_1, w2_1,
    expert_weights, indices0, expert_ids,
) -> list[dict]:
    h = np.asarray(hidden_states, dtype=np.float32)
    ew = np.asarray(expert_weights, dtype=np.float32)
    eid = np.asarray(expert_ids)
    swap = bool(eid[0] != 0)
    ew0 = float(ew[1] if swap else ew[0])
    ew1 = float(ew[0] if swap else ew[1])

    idx = np.asarray(indices0).astype(np.int64)
    idxp = np.concatenate([idx, np.zeros(A_PAD - idx.shape[0], np.int64)])

    def prep_expert(w3, w1, w2, scale):
        w3g = np.asarray(w3, np.float32)[idxp].astype(np.float16)   # [A_PAD, D]
        w1p = np.zeros((A_PAD, D), np.float16)
        w1p[:ACTIVE] = np.asarray(w1, np.float32).astype(np.float16)
        w2p = np.zeros((A_PAD, D), np.float16)
        w2p[:ACTIVE] = (np.asarray(w2, np.float32) * scale).astype(np.float16)
        return w3g, w1p, w2p

    w3g0, w1p0, w2p0 = prep_expert(w3_0, w1_0, w2_0, ew0)
    w3g1, w1p1, w2p1 = prep_expert(w3_1, w1_1, w2_1, ew1)

    hT = np.ascontiguousarray(
        h.T.astype(np.float16).reshape(KCH, 128, T).transpose(1, 0, 2).reshape(128, KCH * T)
    )

    KH = KCH // 2

    def slab(Wrows):  # [mw, D] -> [128, KCH, mw] with [p, k, j]
        mw = Wrows.shape[0]
        return Wrows.T.reshape(KCH, 128, mw).transpose(1, 0, 2)

    in_maps = []
    for c in range(NCORES):
        wug_c = np.empty((128, WUG_COLS), np.float16)
        for e, (w3g, w1p) in enumerate(((w3g0, w1p0), (w3g1, w1p1))):
            for m in range(MCH):
                mw = min(128, AC - m * 128)
                off = _SLAB_OFF[(e, m)]
                r = slice(c * AC + m * 128, c * AC + m * 128 + mw)
                su, sg = slab(w3g[r]), slab(w1p[r])
                # per k-half hh: [u k-half | g k-half], each KH*mw wide
                for hh in range(2):
                    ho = off + hh * KCH * mw
                    wug_c[:, ho: ho + KH * mw] = \
                        su[:, hh * KH:(hh + 1) * KH].reshape(128, KH * mw)
                    wug_c[:, ho + KH * mw: ho + 2 * KH * mw] = \
                        sg[:, hh * KH:(hh + 1) * KH].reshape(128, KH * mw)
        r = slice(c * AC, (c + 1) * AC)
        w2_c = np.ascontiguousarray(np.stack([w2p0[r], w2p1[r]]))  # [2, AC, D]
        in_maps.append({"h": hT, "wug": wug_c, "w2": w2_c})
    return in_maps


def reduce_outputs(results: list[dict]) -> np.ndarray:
    total = np.zeros((T, D), np.float64)
    for res in results:
        x = np.asarray(res["out"])                    # [128, 1024] f32
        total += x.reshape(128, 2, 16, T).transpose(3, 1, 2, 0).reshape(T, D)
    return total.astype(np.float32)


def run_spmd(in_maps, **kwargs):
    nc = get_program()
    return run_bass_kernel_spmd(nc, in_maps, core_ids=list(range(NCORES)), **kwargs)


def kernel(**inputs) -> np.ndarray:
    in_maps = prepare_in_maps(**inputs)
    res = run_spmd(in_maps)
    return reduce_outputs(res.results)


# revision 13
# speedup vs baseline: 2.7181x; 1.0017x over previous
"""Trainium2 Bass kernel for nn_CachedMLP (2-expert cached MoE MLP).

Math (per reference.py): for each expert e in {0,1}
    u_e = (h @ w3_e.T)[:, idx]  ==  h @ (w3_e[idx, :]).T      (column gather == row gather on w3)
    g_e = silu(h @ w1_e.T)
    out = sum_e ew_e * ((g_e * u_e) @ w2_e)

Strategy (memory-bound problem, ~1.2 GB fp32 of weights, 32 tokens):
  * Host: apply the index gather to w3 rows, fold the routing scalars ew_e
    into w2, pad ACTIVE 11468 -> 11472, shard all three weight matrices
    along the ACTIVE axis across 8 NeuronCores, cast to fp16 (halves HBM
    traffic; fp16 keeps ~5e-4 matmul rounding vs bf16's ~4e-3).
  * Device (per core), per (expert, 128-row ACTIVE chunk):
      - DMA one contraction slab (w3gT and w1T k-chunks) and the w2 row
        strip; 32+32 accumulating matmuls -> uT/gT [mw, 32] in PSUM
        (one accumulation group per bank at a time — HW `start` clears
        has_written for the whole bank);
      - silu (ACT) * mul (DVE) -> pT [mw, 32] fp16;
      - 32 single-shot matmuls w2-chunk.T @ pT -> outT n-chunks [128, 32],
        16 per scratch bank (sequential groups; data persists after stop);
      - DVE-accumulate the two scratch banks into an SBUF outT accumulator.
  * Host: un-transpose and sum the 8 per-core partials (no device
    collectives needed).

kernel(**inputs) takes the full unsharded inputs and returns the full
[32, 4096] fp32 output.
"""

import numpy as np

import concourse.bass as bass
import concourse.mybir as mybir
import concourse.tile as tile
from concourse import bacc
from concourse.bass_utils import run_bass_kernel_spmd

NCORES = 8
T = 32              # tokens
D = 4096            # d_model
HIDDEN = 14336
ACTIVE = 11468
A_PAD = 11472       # ACTIVE padded to a multiple of NCORES
AC = A_PAD // NCORES          # 1434 ACTIVE-rows per core
MCH = (AC + 127) // 128       # 12 chunks of <=128 rows (last chunk = 26)
KCH = D // 128                # 32 contraction chunks over d_model
FD = mybir.dt.float16
F32 = mybir.dt.float32

# column widths/offsets of the per-(e,m) slabs inside the packed wug tensor
_SLAB_W = [2 * KCH * min(128, AC - m * 128) for m in range(MCH)]
_SLAB_OFF = {}
_off = 0
for _e in range(2):
    for _m in range(MCH):
        _SLAB_OFF[(_e, _m)] = _off
        _off += _SLAB_W[_m]
WUG_COLS = _off  # 2 * 2*KCH*AC = 183552

_CACHE: dict = {}


def build_program(reps: int = 1, mode: str = "full") -> bass.Bass:
    """mode: 'full' (real kernel), 'dma' (DMAs only), 'pe' (compute only,
    static tiles) — the latter two are bottleneck-attribution diagnostics."""
    do_dma = mode in ("full", "dma")
    do_pe = mode in ("full", "pe")
    nc = bacc.Bacc("TRN2", target_bir_lowering=False, debug=False, num_devices=NCORES)

    h_in = nc.dram_tensor("h", [128, KCH * T], FD, kind="ExternalInput")
    # wug[p, SLAB_OFF(e,m) + which*KCH*mw + k*mw + j] = W.T[k*128 + p, m*128 + j]
    #   W = w3_gathered_e (which=0) or w1_e (which=1), rows local to this shard
    wug = nc.dram_tensor("wug", [128, WUG_COLS], FD, kind="ExternalInput")
    w2 = nc.dram_tensor("w2", [2, AC, D], FD, kind="ExternalInput")
    # out[p, b*512 + nl*32 + t] = outT[(b*16+nl)*128 + p, t]  (partial over shard)
    out = nc.dram_tensor("out", [128, 1024], F32, kind="ExternalOutput")

    AF = mybir.ActivationFunctionType

    KH = KCH // 2  # k-chunks per slab half

    with tile.TileContext(nc) as tc:
        with (
            tc.tile_pool(name="hp", bufs=1) as hp,
            tc.tile_pool(name="slabs", bufs=6) as slabs,
            tc.tile_pool(name="w2pool", bufs=6) as w2pool,
            tc.tile_pool(name="ptp", bufs=3) as ptp,
            tc.tile_pool(name="silp", bufs=3) as silp,
            tc.tile_pool(name="obp", bufs=2) as obp,
            tc.tile_pool(name="pug", bufs=2, space="PSUM") as pug,
            tc.tile_pool(name="pos", bufs=2, space="PSUM") as pos,
        ):
            ht = hp.tile([128, KCH * T], FD, name="ht")
            nc.sync.dma_start(ht[:], h_in[:])

            if not do_dma:  # static operand tiles for the PE-only diagnostic
                sl_static = slabs.tile([128, KCH * 128], FD, name="sl_st", tag="slab")
                nc.gpsimd.memset(sl_static[:], 0.0)
                w2_static = []
                for b in range(2):
                    w2t = w2pool.tile([128, D // 2], FD, name=f"w2_st{b}",
                                      tag=f"w2t{b}")
                    nc.gpsimd.memset(w2t[:], 0.0)
                    w2_static.append(w2t)

            def emit_head(rep, e, m):
                """DMAs + u/g accumulation for one (expert, chunk)."""
                mw = min(128, AC - m * 128)
                off = _SLAB_OFF[(e, m)]
                st = {"mw": mw}

                if do_dma:
                    # two slab halves: [u k-half | g k-half], 1 DMA each
                    sls = []
                    for hh in range(2):
                        slh = slabs.tile([128, KCH * 128], FD,
                                         name=f"sl{rep}_{e}_{m}_{hh}", tag="slab")
                        nc.sync.dma_start(
                            slh[:, : KCH * mw],
                            wug[:, off + hh * KCH * mw: off + (hh + 1) * KCH * mw],
                        )
                        sls.append(slh)
                    # w2 column halves, one per outT scratch bank, on the
                    # second HWDGE ring (scalar) for queue parallelism
                    w2h = []
                    for b in range(2):
                        w2t = w2pool.tile([128, D // 2], FD,
                                          name=f"w2_{rep}_{e}_{m}_{b}",
                                          tag=f"w2t{b}")
                        nc.scalar.dma_start(
                            w2t[:mw],
                            w2[e, m * 128: m * 128 + mw,
                               b * (D // 2): (b + 1) * (D // 2)],
                        )
                        w2h.append(w2t)
                else:
                    sls = [sl_static, sl_static]
                    w2h = w2_static
                st["w2h"] = w2h

                if not do_pe:
                    return st

                accu = pug.tile([128, T], F32, name=f"au{rep}_{e}_{m}", tag="accu")
                accg = pug.tile([128, T], F32, name=f"ag{rep}_{e}_{m}", tag="accg")
                for hh in range(2):
                    for which, acc in ((0, accu), (1, accg)):
                        for kl in range(KH):
                            k = hh * KH + kl
                            nc.tensor.matmul(
                                acc[:mw],
                                lhsT=sls[hh][:, (which * KH + kl) * mw:
                                             (which * KH + kl + 1) * mw],
                                rhs=ht[:, k * T:(k + 1) * T],
                                start=(k == 0), stop=(k == KCH - 1),
                            )
                st["accu"], st["accg"] = accu, accg
                return st

            def emit_tail(rep, e, m, st, osb):
                """silu*mul + outT matmuls + SBUF accumulate for one (e, m).
                Emitted one iteration late so the PE never stalls on the
                ACT/DVE chain that produces pT."""
                if not do_pe:
                    return
                mw = st["mw"]
                accu, accg, w2h = st["accu"], st["accg"], st["w2h"]

                # silu(g) = g * sigmoid(g); Sigmoid is HW-LUT'd and
                # implemented in CoreSim (Silu is not)
                sig = silp.tile([128, T], F32, name=f"sig{rep}_{e}_{m}", tag="sig")
                nc.scalar.activation(sig[:mw], accg[:mw], AF.Sigmoid)
                sil = silp.tile([128, T], F32, name=f"sil{rep}_{e}_{m}", tag="sil")
                nc.vector.tensor_mul(sil[:mw], sig[:mw], accg[:mw])
                pt = ptp.tile([128, T], FD, name=f"pt{rep}_{e}_{m}", tag="pt")
                nc.vector.tensor_mul(pt[:mw], sil[:mw], accu[:mw])

                # outT chunks: 16 sequential single-shot groups per bank
                for b in range(2):
                    osc = pos.tile([128, 512], F32,
                                   name=f"os{rep}_{e}_{m}_{b}", tag=f"osc{b}")
                    for nl in range(16):
                        nc.tensor.matmul(
                            osc[:, nl * T:(nl + 1) * T],
                            lhsT=w2h[b][:mw, nl * 128:(nl + 1) * 128],
                            rhs=pt[:mw],
                            start=True, stop=True,
                        )
                    nc.vector.tensor_add(
                        osb[:, b * 512:(b + 1) * 512],
                        osb[:, b * 512:(b + 1) * 512],
                        osc[:],
                    )

            seq = [(e, m) for e in range(2) for m in range(MCH)]
            for rep in range(reps):
                osb = obp.tile([128, 1024], F32, name=f"osb{rep}", tag="osb")
                nc.gpsimd.memset(osb[:], 0.0)

                state = {}
                for i in range(len(seq) + 1):
                    if i < len(seq):
                        state[i] = emit_head(rep, *seq[i])
                    if i >= 1:
                        emit_tail(rep, *seq[i - 1], state.pop(i - 1), osb)

                nc.sync.dma_start(out[:], osb[:])

    nc.compile()
    return nc


def get_program(reps: int = 1, mode: str = "full") -> bass.Bass:
    key = ("nc", reps, mode)
    if key not in _CACHE:
        _CACHE[key] = build_program(reps, mode)
    return _CACHE[key]


def prepare_in_maps(
    hidden_states, w3_0, w3_1, w1_0, w2_0, w1_1, w2_1,
    expert_weights, indices0, expert_ids,
) -> list[dict]:
    h = np.asarray(hidden_states, dtype=np.float32)
    ew = np.asarray(expert_weights, dtype=np.float32)
    eid = np.asarray(expert_ids)
    swap = bool(eid[0] != 0)
    ew0 = float(ew[1] if swap else ew[0])
    ew1 = float(ew[0] if swap else ew[1])

    idx = np.asarray(indices0).astype(np.int64)
    idxp = np.concatenate([idx, np.zeros(A_PAD - idx.shape[0], np.int64)])

    def prep_expert(w3, w1, w2, scale):
        w3g = np.asarray(w3, np.float32)[idxp].astype(np.float16)   # [A_PAD, D]
        w1p = np.zeros((A_PAD, D), np.float16)
        w1p[:ACTIVE] = np.asarray(w1, np.float32).astype(np.float16)
        w2p = np.zeros((A_PAD, D), np.float16)
        w2p[:ACTIVE] = (np.asarray(w2, np.float32) * scale).astype(np.float16)
        return w3g, w1p, w2p

    w3g0, w1p0, w2p0 = prep_expert(w3_0, w1_0, w2_0, ew0)
    w3g1, w1p1, w2p1 = prep_expert(w3_1, w1_1, w2_1, ew1)

    hT = np.ascontiguousarray(
        h.T.astype(np.float16).reshape(KCH, 128, T).transpose(1, 0, 2).reshape(128, KCH * T)
    )

    KH = KCH // 2

    def slab(Wrows):  # [mw, D] -> [128, KCH, mw] with [p, k, j]
        mw = Wrows.shape[0]
        return Wrows.T.reshape(KCH, 128, mw).transpose(1, 0, 2)

    in_maps = []
    for c in range(NCORES):
        wug_c = np.empty((128, WUG_COLS), np.float16)
        for e, (w3g, w1p) in enumerate(((w3g0, w1p0), (w3g1, w1p1))):
            for m in range(MCH):
                mw = min(128, AC - m * 128)
                off = _SLAB_OFF[(e, m)]
                r = slice(c * AC + m * 128, c * AC + m * 128 + mw)
                su, sg = slab(w3g[r]), slab(w1p[r])
                # per k-half hh: [u k-half | g k-half], each KH*mw wide
                for hh in range(2):
                    ho = off + hh * KCH * mw
                    wug_c[:, ho: ho + KH * mw] = \
                        su[:, hh * KH:(hh + 1) * KH].reshape(128, KH * mw)
                    wug_c[:, ho + KH * mw: ho + 2 * KH * mw] = \
                        sg[:, hh * KH:(hh + 1) * KH].reshape(128, KH * mw)
        r = slice(c * AC, (c + 1) * AC)
        w2_c = np.ascontiguousarray(np.stack([w2p0[r], w2p1[r]]))  # [2, AC, D]
        in_maps.append({"h": hT, "wug": wug_c, "w2": w2_c})
    return in_maps


def reduce_outputs(results: list[dict]) -> np.ndarray:
    total = np.zeros((T, D), np.float64)
    for res in results:
        x = np.asarray(res["out"])                    # [128, 1024] f32
        total += x.reshape(128, 2, 16, T).transpose(3, 1, 2, 0).reshape(T, D)
    return total.astype(np.float32)


def run_spmd(in_maps, **kwargs):
    nc = get_program()
    return run_bass_kernel_spmd(nc, in_maps, core_ids=list(range(NCORES)), **kwargs)


def kernel(**inputs) -> np.ndarray:
    in_maps = prepare_in_maps(**inputs)
    res = run_spmd(in_maps)
    return reduce_outputs(res.results)
